# revision 1
# baseline (speedup 1.0000x reference)
"""Trainium2 Bass kernel for the e3nn-style concat + per-irrep Linear problem.

Reference computation (N = 200000 nodes, 480-dim features per input):
  per input: 128x0e (dims 0:128) + 64x1e (dims 128:320) + 32x2e (dims 320:480)
  s = [s1, s2] @ W0 * inv0 + b0                   # [N, 128]
  v = einsum('nmi,mo->noi', [v1,v2], W1) * inv1   # [N, 64, 3]
  t = einsum('nmi,mo->noi', [t1,t2], W2) * inv2   # [N, 32, 5]
  out = concat([s, v.flat, t.flat], axis=1)       # [N, 480]

Strategy (memory-bound, data-parallel over nodes across 8 cores):
  - The whole pipeline is HBM-bandwidth bound; the harness accuracy gate
    is rel_err < 2e-2, so stream everything as bf16 (quantization error
    ~3e-3) to halve HBM traffic vs fp32.
  - DRAM layout is partition-major per 1000-node block: each SBUF
    partition's slice of a block is one contiguous 6-8 KB run in DRAM,
    so every DMA descriptor is large (smaller descriptors cap each of
    the 16 SDMA engines at ~21 GB/s; 8 KB runs them at line rate).
  - Host: repack both inputs into per-core block tensors (irrep
    components de-interleaved into eight 128-row contraction slabs),
    fold the 1/sqrt(K) norms into the bf16 weights, pair up the five
    l=2 components into block-diagonal weights. 25000 nodes/core =
    25 blocks x 1000 -> zero padding.
  - Device: per block, three HWDGE loads (1.0 + 0.75 + 0.125 MB), 18
    bf16 matmuls (two 500-column chunks; all four output row-chunks of
    a slice land in one 4-bank PSUM tile, bias via a K=1 ones-vector
    matmul accumulate), one strided DVE copy per chunk (fp32 PSUM ->
    bf16 SBUF), one SWDGE store per block.
  - Host: transpose/interleave the bf16 out tensor back to the fp32
    reference layout.
"""

import numpy as np
import ml_dtypes

BF16 = ml_dtypes.bfloat16
MUL0, MUL1, MUL2 = 128, 64, 32
N_TOTAL = 200000
N_CORES = 8
NC_NODES = N_TOTAL // N_CORES          # 25000
NODE_BLOCK = 1000
HALF = NODE_BLOCK // 2                 # 500-col matmul chunks (fp32 PSUM bank)
# Last 2000 nodes run as four 500-node half-blocks: the end-of-stream drain
# (compute+store after the final load) works in half-size quanta.
N_BLOCKS = 23                          # full 1000-node blocks
N_TBLOCKS = 4                          # tail half-blocks of HALF nodes
MAIN_NODES = N_BLOCKS * NODE_BLOCK     # 23000
NPAD = NC_NODES                        # no padding anywhere

_PROGRAM_CACHE = {}


def _build_program():
    import concourse.mybir as mybir
    from concourse import bacc
    import concourse.tile as tile

    f32 = mybir.dt.float32
    bf16 = mybir.dt.bfloat16
    NB = NODE_BLOCK
    nc = bacc.Bacc("TRN2", target_bir_lowering=False, debug=False)

    nblocks = N_BLOCKS
    # Partition-major block layouts: row p of block b is p's whole SBUF
    # line, so each DMA descriptor is one contiguous 2*k*NB-byte run.
    xina = nc.dram_tensor("xina", [nblocks, 128, 4 * NB], bf16, kind="ExternalInput").ap()
    xinb = nc.dram_tensor("xinb", [nblocks, 128, 3 * NB], bf16, kind="ExternalInput").ap()
    xinc = nc.dram_tensor("xinc", [nblocks, 64, NB], bf16, kind="ExternalInput").ap()
    xina2 = nc.dram_tensor("xina2", [N_TBLOCKS, 128, 4 * HALF], bf16, kind="ExternalInput").ap()
    xinb2 = nc.dram_tensor("xinb2", [N_TBLOCKS, 128, 3 * HALF], bf16, kind="ExternalInput").ap()
    xinc2 = nc.dram_tensor("xinc2", [N_TBLOCKS, 64, HALF], bf16, kind="ExternalInput").ap()
    w0a = nc.dram_tensor("w0a", [128, 128], bf16, kind="ExternalInput").ap()
    w0b = nc.dram_tensor("w0b", [128, 128], bf16, kind="ExternalInput").ap()
    w1d = nc.dram_tensor("w1d", [128, 64], bf16, kind="ExternalInput").ap()
    w2p = nc.dram_tensor("w2p", [128, 64], bf16, kind="ExternalInput").ap()
    w2s = nc.dram_tensor("w2s", [64, 32], bf16, kind="ExternalInput").ap()
    # bias as a K=1 matmul operand: lhsT [1,128] = b0, rhs [1,NB] = ones
    b0d = nc.dram_tensor("b0d", [1, 128], bf16, kind="ExternalInput").ap()
    oned = nc.dram_tensor("oned", [1, NB], bf16, kind="ExternalInput").ap()
    # Single store tensor; rows 96:128 of the 4th chunk are junk (never read
    # back) — one 8KB-per-partition store descriptor beats two DMAs.
    outa = nc.dram_tensor("outa", [nblocks, 128, 4 * NB], bf16, kind="ExternalOutput").ap()
    outa2 = nc.dram_tensor("outa2", [N_TBLOCKS, 128, 4 * HALF], bf16, kind="ExternalOutput").ap()

    with tile.TileContext(nc) as tc:
        with (
            tc.tile_pool(name="wpool", bufs=1) as wpool,
            # bufs=3 balances the start-of-pipe load stall (deeper is better)
            # against the end-of-stream store backlog (shallower is better);
            # 2 measurably worsens the start, 6 measurably lengthens the tail.
            tc.tile_pool(name="inpool", bufs=3) as inpool,
            tc.tile_pool(name="psum", bufs=2, space="PSUM") as psum,
            tc.tile_pool(name="outpool", bufs=3) as outpool,
        ):
            wa_t = wpool.tile([128, 128], bf16)
            wb_t = wpool.tile([128, 128], bf16)
            w1_t = wpool.tile([128, 64], bf16)
            w2p_t = wpool.tile([128, 64], bf16)
            w2s_t = wpool.tile([64, 32], bf16)
            b0_t = wpool.tile([1, 128], bf16)
            one_t = wpool.tile([1, NB], bf16)
            # Weights ride the SWDGE ring so the HWDGE rings start streaming
            # block loads immediately.
            nc.gpsimd.dma_start(wa_t[:], w0a)
            nc.gpsimd.dma_start(wb_t[:], w0b)
            nc.gpsimd.dma_start(w1_t[:], w1d)
            nc.gpsimd.dma_start(w2p_t[:], w2p)
            nc.gpsimd.dma_start(w2s_t[:], w2s)
            nc.gpsimd.dma_start(b0_t[:], b0d)
            nc.gpsimd.dma_start(one_t[:], oned)

            for blk in range(nblocks):
                # Loads split across both HWDGE rings (qSP gets the
                # [s1|s2|v0|v1] tile; qACT gets [v2|tp0|tp1] + [t4]) so the
                # rings stream concurrently and loads never queue behind
                # stores. Stores go through SWDGE (gpsimd).
                tina = inpool.tile([128, 4 * NB], bf16)
                tinb = inpool.tile([128, 3 * NB], bf16)
                t4t = inpool.tile([64, NB], bf16)
                nc.sync.dma_start(tina[:], xina[blk])
                nc.scalar.dma_start(tinb[:], xinb[blk])
                # alternate the small t4 load between the rings for balance
                (nc.sync if blk % 2 == 0 else nc.scalar).dma_start(t4t[:], xinc[blk])

                # out row chunks: [s(128)] [v0|v1] [v2|t0,t1] [t2,t3|t4] (96)
                tout = outpool.tile([128, 4 * NB], bf16)
                for h in range(2):
                    o = h * HALF
                    # All four row chunks of this 500-col slice in one PSUM
                    # tile spanning 4 banks (regions at 512-col boundaries),
                    # so a single strided DVE copy drains the whole chunk.
                    # Bias lands via a K=1 ones-vector matmul accumulate.
                    pbig = psum.tile([128, 2048], f32)

                    nc.tensor.matmul(pbig[:, 0:HALF], wa_t[:], tina[:, o:o + HALF],
                                     start=True, stop=False)
                    nc.tensor.matmul(pbig[:, 0:HALF], wb_t[:], tina[:, NB + o:NB + o + HALF],
                                     start=False, stop=False)
                    nc.tensor.matmul(pbig[:, 0:HALF], b0_t[:], one_t[:, o:o + HALF],
                                     start=False, stop=True)
                    nc.tensor.matmul(pbig[0:64, 512:512 + HALF], w1_t[:], tina[:, 2 * NB + o:2 * NB + o + HALF])
                    nc.tensor.matmul(pbig[64:128, 512:512 + HALF], w1_t[:], tina[:, 3 * NB + o:3 * NB + o + HALF])
                    nc.tensor.matmul(pbig[0:64, 1024:1024 + HALF], w1_t[:], tinb[:, o:o + HALF])
                    nc.tensor.matmul(pbig[64:128, 1024:1024 + HALF], w2p_t[:], tinb[:, NB + o:NB + o + HALF])
                    nc.tensor.matmul(pbig[0:64, 1536:1536 + HALF], w2p_t[:], tinb[:, 2 * NB + o:2 * NB + o + HALF])
                    nc.tensor.matmul(pbig[64:96, 1536:1536 + HALF], w2s_t[:], t4t[:, o:o + HALF])

                    nc.vector.tensor_copy(
                        tout[:].rearrange("p (c n) -> p c n", n=NB)[:, :, o:o + HALF],
                        pbig[:].rearrange("p (c n) -> p c n", n=512)[:, :, 0:HALF],
                    )

                nc.gpsimd.dma_start(outa[blk], tout[:])

            for u in range(N_TBLOCKS):
                tina2 = inpool.tile([128, 4 * HALF], bf16)
                tinb2 = inpool.tile([128, 3 * HALF], bf16)
                t4t2 = inpool.tile([64, HALF], bf16)
                nc.sync.dma_start(tina2[:], xina2[u])
                nc.scalar.dma_start(tinb2[:], xinb2[u])
                (nc.sync if u % 2 == 0 else nc.scalar).dma_start(t4t2[:], xinc2[u])

                tout2 = outpool.tile([128, 4 * HALF], bf16)
                pbig = psum.tile([128, 2048], f32)

                nc.tensor.matmul(pbig[:, 0:HALF], wa_t[:], tina2[:, 0:HALF],
                                 start=True, stop=False)
                nc.tensor.matmul(pbig[:, 0:HALF], wb_t[:], tina2[:, HALF:2 * HALF],
                                 start=False, stop=False)
                nc.tensor.matmul(pbig[:, 0:HALF], b0_t[:], one_t[:, 0:HALF],
                                 start=False, stop=True)
                nc.tensor.matmul(pbig[0:64, 512:512 + HALF], w1_t[:], tina2[:, 2 * HALF:3 * HALF])
                nc.tensor.matmul(pbig[64:128, 512:512 + HALF], w1_t[:], tina2[:, 3 * HALF:4 * HALF])
                nc.tensor.matmul(pbig[0:64, 1024:1024 + HALF], w1_t[:], tinb2[:, 0:HALF])
                nc.tensor.matmul(pbig[64:128, 1024:1024 + HALF], w2p_t[:], tinb2[:, HALF:2 * HALF])
                nc.tensor.matmul(pbig[0:64, 1536:1536 + HALF], w2p_t[:], tinb2[:, 2 * HALF:3 * HALF])
                nc.tensor.matmul(pbig[64:96, 1536:1536 + HALF], w2s_t[:], t4t2[:])

                nc.vector.tensor_copy(
                    tout2[:].rearrange("p (c n) -> p c n", n=HALF),
                    pbig[:].rearrange("p (c n) -> p c n", n=512)[:, :, 0:HALF],
                )
                nc.gpsimd.dma_start(outa2[u], tout2[:])

    nc.compile()
    return nc


def _get_program(key="bf16"):
    key = "bf16"
    if key not in _PROGRAM_CACHE:
        _PROGRAM_CACHE[key] = _build_program()
    return _PROGRAM_CACHE[key]


def _repack_inputs(x1, x2):
    """Build the eight 128-row contraction slabs [960, N] in bf16.

    Slabs: [s1] [s2] [v1_0|v2_0] [v1_1|v2_1] [v1_2|v2_2]
    [t_0|t_1] [t_2|t_3] [t_4], each t_i = [t1_i(32); t2_i(32)].
    """
    n = x1.shape[0]
    x1b = x1.astype(BF16)
    x2b = x2.astype(BF16)
    xr = np.empty((960, n), dtype=BF16)
    xr[0:128] = x1b[:, 0:128].T
    xr[128:256] = x2b[:, 0:128].T
    v1 = x1b[:, 128:320].reshape(n, MUL1, 3)
    v2 = x2b[:, 128:320].reshape(n, MUL1, 3)
    for i in range(3):
        base = 256 + 128 * i
        xr[base:base + 64] = v1[:, :, i].T
        xr[base + 64:base + 128] = v2[:, :, i].T
    t1 = x1b[:, 320:480].reshape(n, MUL2, 5)
    t2 = x2b[:, 320:480].reshape(n, MUL2, 5)
    for i in range(5):
        base = 640 + 64 * i
        xr[base:base + 32] = t1[:, :, i].T
        xr[base + 32:base + 64] = t2[:, :, i].T
    return xr


def _to_pmajor(sl, nrows, nchunks, nblocks, nb):
    """[nchunks*nrows, nblocks*nb] slab-major -> [nblocks, nrows, nchunks*nb] p-major."""
    # sl[c*nrows + p, b*nb + j] -> out[b, p, c*nb + j]
    return np.ascontiguousarray(
        sl.reshape(nchunks, nrows, nblocks, nb).transpose(2, 1, 0, 3)
        .reshape(nblocks, nrows, nchunks * nb)
    )


def _prepare_in_maps(x1, x2, W0, W1, W2, b0):
    x1 = np.asarray(x1, dtype=np.float32)
    x2 = np.asarray(x2, dtype=np.float32)
    inv0 = np.float32(1.0 / np.sqrt(2 * MUL0))
    inv1 = np.float32(1.0 / np.sqrt(2 * MUL1))
    inv2 = np.float32(1.0 / np.sqrt(2 * MUL2))
    w0s = np.asarray(W0, np.float32) * inv0                            # [256, 128]
    w1s = np.asarray(W1, np.float32) * inv1                            # [128, 64]
    w2s = np.asarray(W2, np.float32) * inv2                            # [64, 32]
    w2pair = np.zeros((128, 64), dtype=np.float32)                     # blockdiag(W2s, W2s)
    w2pair[0:64, 0:32] = w2s
    w2pair[64:128, 32:64] = w2s
    weights = {
        "w0a": np.ascontiguousarray(w0s[0:128]).astype(BF16),
        "w0b": np.ascontiguousarray(w0s[128:256]).astype(BF16),
        "w1d": np.ascontiguousarray(w1s).astype(BF16),
        "w2p": w2pair.astype(BF16),
        "w2s": np.ascontiguousarray(w2s).astype(BF16),
        "b0d": np.asarray(b0, np.float32).reshape(1, 128).astype(BF16),
        "oned": np.ones((1, NODE_BLOCK), dtype=BF16),
    }
    xr = _repack_inputs(x1, x2)
    in_maps = []
    for c in range(N_CORES):
        xc = xr[:, c * NC_NODES:(c + 1) * NC_NODES]
        xm, xt = xc[:, 0:MAIN_NODES], xc[:, MAIN_NODES:]
        in_maps.append({
            "xina": _to_pmajor(xm[0:512], 128, 4, N_BLOCKS, NODE_BLOCK),
            "xinb": _to_pmajor(xm[512:896], 128, 3, N_BLOCKS, NODE_BLOCK),
            "xinc": _to_pmajor(xm[896:960], 64, 1, N_BLOCKS, NODE_BLOCK),
            "xina2": _to_pmajor(xt[0:512], 128, 4, N_TBLOCKS, HALF),
            "xinb2": _to_pmajor(xt[512:896], 128, 3, N_TBLOCKS, HALF),
            "xinc2": _to_pmajor(xt[896:960], 64, 1, N_TBLOCKS, HALF),
            **weights,
        })
    return in_maps


def _from_pmajor(oa, nblocks, nb):
    """[nblocks,128,4*nb] p-major -> [480, nblocks*nb] slab-major (junk rows
    96:128 of chunk 3 dropped)."""
    oa = np.asarray(oa).reshape(nblocks, 128, 4, nb)
    o3 = (oa[:, :, 0:3, :].transpose(2, 1, 0, 3)
          .reshape(3 * 128, nblocks * nb).astype(np.float32))
    ob = (oa[:, 0:96, 3, :].transpose(1, 0, 2)
          .reshape(96, nblocks * nb).astype(np.float32))
    return np.concatenate([o3, ob], axis=0)


def _assemble_output(outs):
    """outs: list of 8 (outa [23,128,4000], outa2 [4,128,2000]) -> [N_TOTAL, 480]."""
    full = np.empty((N_TOTAL, 480), dtype=np.float32)
    for c, (oa, oa2) in enumerate(outs):
        o = np.concatenate([
            _from_pmajor(oa, N_BLOCKS, NODE_BLOCK),
            _from_pmajor(oa2, N_TBLOCKS, HALF),
        ], axis=1)                                 # [480, NC_NODES] slab-major
        rows = slice(c * NC_NODES, (c + 1) * NC_NODES)
        full[rows, 0:128] = o[0:128].T
        full[rows, 128:320] = (
            o[128:320].reshape(3, MUL1, NC_NODES).transpose(2, 1, 0).reshape(NC_NODES, 192)
        )
        full[rows, 320:480] = (
            o[320:480].reshape(5, MUL2, NC_NODES).transpose(2, 1, 0).reshape(NC_NODES, 160)
        )
    return full


def kernel(x1, x2, W0, W1, W2, b0):
    from concourse.bass_utils import run_bass_kernel_spmd

    in_maps = _prepare_in_maps(x1, x2, W0, W1, W2, b0)
    nc = _get_program()
    res = run_bass_kernel_spmd(nc, in_maps, core_ids=list(range(N_CORES)))
    return _assemble_output([(r["outa"], r["outa2"]) for r in res.results])



# revision 6
# speedup vs baseline: 1.0891x; 1.0891x over previous
"""Trainium2 Bass kernel for the e3nn-style concat + per-irrep Linear problem.

Reference computation (N = 200000 nodes, 480-dim features per input):
  per input: 128x0e (dims 0:128) + 64x1e (dims 128:320) + 32x2e (dims 320:480)
  s = [s1, s2] @ W0 * inv0 + b0                   # [N, 128]
  v = einsum('nmi,mo->noi', [v1,v2], W1) * inv1   # [N, 64, 3]
  t = einsum('nmi,mo->noi', [t1,t2], W2) * inv2   # [N, 32, 5]
  out = concat([s, v.flat, t.flat], axis=1)       # [N, 480]

Strategy (memory-bound, data-parallel over nodes across 8 cores):
  - The whole pipeline is HBM-bandwidth bound; the harness accuracy gate
    is rel_err < 2e-2, so stream everything as bf16 (quantization error
    ~3e-3) to halve HBM traffic vs fp32.
  - DRAM layout is partition-major per 1000-node block: each SBUF
    partition's slice of a block is one contiguous 7-8 KB run in DRAM,
    so every DMA descriptor is large.
  - 25 UNIFORM blocks of 1000 nodes per core (25000 nodes/core). The
    previous revision used 4 half-size tail blocks with different tile
    shapes; the shape change flushed the tile pools and cost a ~20us
    load stall at the main->tail transition.
  - The small t4 slab ([64, NB]) is folded into the xinb tile as a
    [128, NB/2] region (partition p carries t4[p%64, (p//64)*500:...]),
    so each block is exactly 2 loads (1.0 MB + 0.875 MB) + 1 store.
    The two loads alternate between the two HWDGE rings per block.
  - Device: per block, 18 bf16 matmuls (two 500-column chunks; all four
    output row-chunks of a slice land in one 4-bank PSUM tile, bias via
    a K=1 ones-vector matmul accumulate), one strided DVE copy per
    chunk (fp32 PSUM -> bf16 SBUF), one SWDGE store per block.
  - Host: transpose/interleave the bf16 out tensor back to the fp32
    reference layout.
"""

import numpy as np
import ml_dtypes

BF16 = ml_dtypes.bfloat16
MUL0, MUL1, MUL2 = 128, 64, 32
N_TOTAL = 200000
N_CORES = 8
NC_NODES = N_TOTAL // N_CORES          # 25000
NODE_BLOCK = 1000
HALF = NODE_BLOCK // 2                 # 500-col matmul chunks (fp32 PSUM bank)
N_BLOCKS = NC_NODES // NODE_BLOCK      # 25 uniform blocks, no padding
NPAD = NC_NODES

_PROGRAM_CACHE = {}


def _build_program():
    import concourse.mybir as mybir
    from concourse import bacc
    import concourse.tile as tile

    f32 = mybir.dt.float32
    bf16 = mybir.dt.bfloat16
    NB = NODE_BLOCK
    nc = bacc.Bacc("TRN2", target_bir_lowering=False, debug=False)

    nblocks = N_BLOCKS
    # Partition-major block layouts: row p of block b is p's whole SBUF
    # line, so each DMA descriptor is one contiguous 7-8 KB run.
    # xina: slabs [s1|s2|v0|v1]; xinb: [v2|t01|t23|t4fold] where t4fold is
    # the [64, NB] t4 slab refolded to [128, NB/2].
    xina = nc.dram_tensor("xina", [nblocks, 128, 4 * NB], bf16, kind="ExternalInput").ap()
    xinb = nc.dram_tensor("xinb", [nblocks, 128, 3 * NB + HALF], bf16, kind="ExternalInput").ap()
    w0a = nc.dram_tensor("w0a", [128, 128], bf16, kind="ExternalInput").ap()
    w0b = nc.dram_tensor("w0b", [128, 128], bf16, kind="ExternalInput").ap()
    w1d = nc.dram_tensor("w1d", [128, 64], bf16, kind="ExternalInput").ap()
    w2p = nc.dram_tensor("w2p", [128, 64], bf16, kind="ExternalInput").ap()
    # w2s duplicated into both partition halves so the t4 matmul's lhsT can
    # start at partition 0 (h=0) or 64 (h=1), matching the folded rhs.
    w2s = nc.dram_tensor("w2s", [128, 32], bf16, kind="ExternalInput").ap()
    # bias as a K=1 matmul operand: lhsT [1,128] = b0, rhs [1,NB] = ones
    b0d = nc.dram_tensor("b0d", [1, 128], bf16, kind="ExternalInput").ap()
    oned = nc.dram_tensor("oned", [1, NB], bf16, kind="ExternalInput").ap()
    # Single store tensor; rows 96:128 of the 4th chunk are junk (never read
    # back) — one 8KB-per-partition store descriptor beats two DMAs.
    outa = nc.dram_tensor("outa", [nblocks, 128, 4 * NB], bf16, kind="ExternalOutput").ap()

    with tile.TileContext(nc) as tc:
        with (
            tc.tile_pool(name="wpool", bufs=1) as wpool,
            tc.tile_pool(name="inpool", bufs=4) as inpool,
            tc.tile_pool(name="psum", bufs=2, space="PSUM") as psum,
            tc.tile_pool(name="outpool", bufs=4) as outpool,
        ):
            wa_t = wpool.tile([128, 128], bf16)
            wb_t = wpool.tile([128, 128], bf16)
            w1_t = wpool.tile([128, 64], bf16)
            w2p_t = wpool.tile([128, 64], bf16)
            w2s_t = wpool.tile([128, 32], bf16)
            b0_t = wpool.tile([1, 128], bf16)
            one_t = wpool.tile([1, NB], bf16)
            # Weights ride the SWDGE ring so the HWDGE rings start streaming
            # block loads immediately.
            nc.gpsimd.dma_start(wa_t[:], w0a)
            nc.gpsimd.dma_start(wb_t[:], w0b)
            nc.gpsimd.dma_start(w1_t[:], w1d)
            nc.gpsimd.dma_start(w2p_t[:], w2p)
            nc.gpsimd.dma_start(w2s_t[:], w2s)
            nc.gpsimd.dma_start(b0_t[:], b0d)
            nc.gpsimd.dma_start(one_t[:], oned)

            for blk in range(nblocks):
                # Two loads per block, alternating HWDGE rings so both rings
                # carry ~the same bytes. Stores go through SWDGE (gpsimd).
                tina = inpool.tile([128, 4 * NB], bf16)
                tinb = inpool.tile([128, 3 * NB + HALF], bf16)
                qa, qb = (nc.sync, nc.scalar) if blk % 2 == 0 else (nc.scalar, nc.sync)
                qa.dma_start(tina[:], xina[blk])
                qb.dma_start(tinb[:], xinb[blk])

                # out row chunks: [s(128)] [v0|v1] [v2|t0,t1] [t2,t3|t4] (96)
                tout = outpool.tile([128, 4 * NB], bf16)
                for h in range(2):
                    o = h * HALF
                    # All four row chunks of this 500-col slice in one PSUM
                    # tile spanning 4 banks (regions at 512-col boundaries),
                    # so a single strided DVE copy drains the whole chunk.
                    # Bias lands via a K=1 ones-vector matmul accumulate.
                    pbig = psum.tile([128, 2048], f32)

                    nc.tensor.matmul(pbig[:, 0:HALF], wa_t[:], tina[:, o:o + HALF],
                                     start=True, stop=False)
                    nc.tensor.matmul(pbig[:, 0:HALF], wb_t[:], tina[:, NB + o:NB + o + HALF],
                                     start=False, stop=False)
                    nc.tensor.matmul(pbig[:, 0:HALF], b0_t[:], one_t[:, o:o + HALF],
                                     start=False, stop=True)
                    nc.tensor.matmul(pbig[0:64, 512:512 + HALF], w1_t[:], tina[:, 2 * NB + o:2 * NB + o + HALF])
                    nc.tensor.matmul(pbig[64:128, 512:512 + HALF], w1_t[:], tina[:, 3 * NB + o:3 * NB + o + HALF])
                    nc.tensor.matmul(pbig[0:64, 1024:1024 + HALF], w1_t[:], tinb[:, o:o + HALF])
                    nc.tensor.matmul(pbig[64:128, 1024:1024 + HALF], w2p_t[:], tinb[:, NB + o:NB + o + HALF])
                    nc.tensor.matmul(pbig[0:64, 1536:1536 + HALF], w2p_t[:], tinb[:, 2 * NB + o:2 * NB + o + HALF])
                    nc.tensor.matmul(pbig[64:96, 1536:1536 + HALF],
                                     w2s_t[h * 64:h * 64 + 64, :],
                                     tinb[h * 64:h * 64 + 64, 3 * NB:3 * NB + HALF])

                    nc.vector.tensor_copy(
                        tout[:].rearrange("p (c n) -> p c n", n=NB)[:, :, o:o + HALF],
                        pbig[:].rearrange("p (c n) -> p c n", n=512)[:, :, 0:HALF],
                    )

                nc.gpsimd.dma_start(outa[blk], tout[:])

    nc.compile()
    return nc


def _get_program(key="bf16"):
    key = "bf16"
    if key not in _PROGRAM_CACHE:
        _PROGRAM_CACHE[key] = _build_program()
    return _PROGRAM_CACHE[key]


def _repack_inputs(x1, x2):
    """Build the eight 128-row contraction slabs [960, N] in bf16.

    Slabs: [s1] [s2] [v1_0|v2_0] [v1_1|v2_1] [v1_2|v2_2]
    [t_0|t_1] [t_2|t_3] [t_4], each t_i = [t1_i(32); t2_i(32)].
    """
    n = x1.shape[0]
    x1b = x1.astype(BF16)
    x2b = x2.astype(BF16)
    xr = np.empty((960, n), dtype=BF16)
    xr[0:128] = x1b[:, 0:128].T
    xr[128:256] = x2b[:, 0:128].T
    v1 = x1b[:, 128:320].reshape(n, MUL1, 3)
    v2 = x2b[:, 128:320].reshape(n, MUL1, 3)
    for i in range(3):
        base = 256 + 128 * i
        xr[base:base + 64] = v1[:, :, i].T
        xr[base + 64:base + 128] = v2[:, :, i].T
    t1 = x1b[:, 320:480].reshape(n, MUL2, 5)
    t2 = x2b[:, 320:480].reshape(n, MUL2, 5)
    for i in range(5):
        base = 640 + 64 * i
        xr[base:base + 32] = t1[:, :, i].T
        xr[base + 32:base + 64] = t2[:, :, i].T
    return xr


def _to_pmajor(sl, nrows, nchunks, nblocks, nb):
    """[nchunks*nrows, nblocks*nb] slab-major -> [nblocks, nrows, nchunks*nb] p-major."""
    # sl[c*nrows + p, b*nb + j] -> out[b, p, c*nb + j]
    return np.ascontiguousarray(
        sl.reshape(nchunks, nrows, nblocks, nb).transpose(2, 1, 0, 3)
        .reshape(nblocks, nrows, nchunks * nb)
    )


def _prepare_in_maps(x1, x2, W0, W1, W2, b0):
    x1 = np.asarray(x1, dtype=np.float32)
    x2 = np.asarray(x2, dtype=np.float32)
    inv0 = np.float32(1.0 / np.sqrt(2 * MUL0))
    inv1 = np.float32(1.0 / np.sqrt(2 * MUL1))
    inv2 = np.float32(1.0 / np.sqrt(2 * MUL2))
    w0s = np.asarray(W0, np.float32) * inv0                            # [256, 128]
    w1s = np.asarray(W1, np.float32) * inv1                            # [128, 64]
    w2s = np.asarray(W2, np.float32) * inv2                            # [64, 32]
    w2pair = np.zeros((128, 64), dtype=np.float32)                     # blockdiag(W2s, W2s)
    w2pair[0:64, 0:32] = w2s
    w2pair[64:128, 32:64] = w2s
    weights = {
        "w0a": np.ascontiguousarray(w0s[0:128]).astype(BF16),
        "w0b": np.ascontiguousarray(w0s[128:256]).astype(BF16),
        "w1d": np.ascontiguousarray(w1s).astype(BF16),
        "w2p": w2pair.astype(BF16),
        "w2s": np.concatenate([w2s, w2s], axis=0).astype(BF16),
        "b0d": np.asarray(b0, np.float32).reshape(1, 128).astype(BF16),
        "oned": np.ones((1, NODE_BLOCK), dtype=BF16),
    }
    xr = _repack_inputs(x1, x2)
    NB = NODE_BLOCK
    in_maps = []
    for c in range(N_CORES):
        xc = xr[:, c * NC_NODES:(c + 1) * NC_NODES]
        xina = _to_pmajor(xc[0:512], 128, 4, N_BLOCKS, NB)
        # xinb = [v2|t01|t23] (3 chunks of 128 rows) + t4 folded to [128, NB/2]
        xinb = np.empty((N_BLOCKS, 128, 3 * NB + HALF), dtype=BF16)
        xinb[:, :, 0:3 * NB] = _to_pmajor(xc[512:896], 128, 3, N_BLOCKS, NB)
        t4 = xc[896:960].reshape(64, N_BLOCKS, NB)                     # [64, blk, NB]
        xinb[:, 0:64, 3 * NB:] = t4[:, :, 0:HALF].transpose(1, 0, 2)
        xinb[:, 64:128, 3 * NB:] = t4[:, :, HALF:].transpose(1, 0, 2)
        in_maps.append({"xina": xina, "xinb": xinb, **weights})
    return in_maps


def _from_pmajor(oa, nblocks, nb):
    """[nblocks,128,4*nb] p-major -> [480, nblocks*nb] slab-major (junk rows
    96:128 of chunk 3 dropped)."""
    oa = np.asarray(oa).reshape(nblocks, 128, 4, nb)
    o3 = (oa[:, :, 0:3, :].transpose(2, 1, 0, 3)
          .reshape(3 * 128, nblocks * nb).astype(np.float32))
    ob = (oa[:, 0:96, 3, :].transpose(1, 0, 2)
          .reshape(96, nblocks * nb).astype(np.float32))
    return np.concatenate([o3, ob], axis=0)


def _assemble_output(outs):
    """outs: list of 8 outa [25,128,4000] -> [N_TOTAL, 480]."""
    full = np.empty((N_TOTAL, 480), dtype=np.float32)
    for c, oa in enumerate(outs):
        o = _from_pmajor(oa, N_BLOCKS, NODE_BLOCK)     # [480, NC_NODES] slab-major
        rows = slice(c * NC_NODES, (c + 1) * NC_NODES)
        full[rows, 0:128] = o[0:128].T
        full[rows, 128:320] = (
            o[128:320].reshape(3, MUL1, NC_NODES).transpose(2, 1, 0).reshape(NC_NODES, 192)
        )
        full[rows, 320:480] = (
            o[320:480].reshape(5, MUL2, NC_NODES).transpose(2, 1, 0).reshape(NC_NODES, 160)
        )
    return full


def kernel(x1, x2, W0, W1, W2, b0):
    from concourse.bass_utils import run_bass_kernel_spmd

    in_maps = _prepare_in_maps(x1, x2, W0, W1, W2, b0)
    nc = _get_program()
    res = run_bass_kernel_spmd(nc, in_maps, core_ids=list(range(N_CORES)))
    return _assemble_output([r["outa"] for r in res.results])
